# revision 1
# baseline (speedup 1.0000x reference)
"""DeepSeekV3 MoE router on 8 TRN2 NeuronCores (Bass/Tile).

Strategy (hardcoded for T=8192, D=7168, E=256, top-k=8, 8 groups, top-4 groups):
  - Data-parallel over tokens: each of 8 cores handles 1024 tokens.
  - Router weight kernel_DE and bias replicated to every core.
  - Host pre-arranges x into the lhsT chunk layout the PE needs
    (contraction dim D on partitions), so no on-chip transposes.
  - Per 128-token tile: 56 accumulating fp32 matmuls -> PSUM scores,
    sigmoid on ACT, grouped top-2 / top-4-groups / top-8 with DVE
    Max8 / max_index / match_replace ops, normalize, DMA out.
"""

import sys

for p in ("/opt/trn_rl_repo", "/root/.axon_site/_ro/trn_rl_repo"):
    if p not in sys.path:
        sys.path.insert(0, p)

import numpy as np

T = 8192
D = 7168
E = 256
TOP_K = 8
N_GROUPS = 8
EPG = E // N_GROUPS  # experts per group = 32
TOPK_GROUPS = 4
SCALE = 2.5
N_CORES = 8
TPC = T // N_CORES  # tokens per core = 1024
N_TILES = TPC // 128  # 8 token tiles per core
KC = D // 128  # 56 contraction chunks

_CACHE = {}


def _build_nc():
    import concourse.bacc as bacc
    import concourse.mybir as mybir
    import concourse.tile as tile

    f32 = mybir.dt.float32
    u32 = mybir.dt.uint32
    X = mybir.AxisListType.X
    Alu = mybir.AluOpType

    nc = bacc.Bacc(trn_type="TRN2")
    x_d = nc.declare_dram_parameter("x", [128, N_TILES, KC, 128], f32, isOutput=False)
    w_d = nc.declare_dram_parameter("w", [128, KC, E], f32, isOutput=False)
    b_d = nc.declare_dram_parameter("bias", [128, E], f32, isOutput=False)
    ow_d = nc.declare_dram_parameter("out_w", [N_TILES, 128, TOP_K], f32, isOutput=True)
    oi_d = nc.declare_dram_parameter("out_idx", [N_TILES, 128, TOP_K], u32, isOutput=True)

    with tile.TileContext(nc) as tc:
        with (
            tc.tile_pool(name="const", bufs=1) as cpool,
            tc.tile_pool(name="xin", bufs=2) as xpool,
            tc.tile_pool(name="xfirst", bufs=1) as xfpool,
            tc.tile_pool(name="work", bufs=2) as wpool,
            tc.tile_pool(name="small", bufs=2) as spool,
            tc.tile_pool(name="psum", bufs=2, space="PSUM") as ppool,
        ):
            # W and x are loaded as SEPARATE tiles per chunk-group so Tile's
            # per-tile dependency tracking lets tile 0's first matmuls start
            # as soon as the first small groups land (instead of stalling on
            # the whole 7.3MB W + 3.7MB x transfer). Graduated piece sizes:
            # tiny leading pieces get the PE started early, then the DMA
            # streams the rest while the PE consumes k in order.
            WPIECES = [2, 2, 3, 7, 14, 14, 14]  # sums to KC=56
            XPIECES = [2, 2, 3, 7, 14, 14, 14]
            XG_STEADY = [14, 14, 14, 14]  # steady-state tiles use 4 pieces
            wk0 = [sum(WPIECES[:i]) for i in range(len(WPIECES))]
            xk0_first = [sum(XPIECES[:i]) for i in range(len(XPIECES))]
            xk0_steady = [sum(XG_STEADY[:i]) for i in range(len(XG_STEADY))]

            def piece_of(k, starts):
                for i in range(len(starts) - 1, -1, -1):
                    if k >= starts[i]:
                        return i, k - starts[i]
                raise AssertionError

            bias_sb = cpool.tile([128, E], f32)
            nc.sync.dma_start(bias_sb[:], b_d[:])
            w_gs = [
                cpool.tile([128, WPIECES[g], E], f32, tag=f"w{g}", name=f"w{g}")
                for g in range(len(WPIECES))
            ]
            x0_ps = [
                xfpool.tile([128, XPIECES[g], 128], f32, tag=f"xf{g}", name=f"x0p{g}")
                for g in range(len(XPIECES))
            ]
            # Interleave W / tile-0 x pieces in k-consumption order.
            for g in range(len(WPIECES)):
                nc.sync.dma_start(
                    w_gs[g][:], w_d[:, wk0[g] : wk0[g] + WPIECES[g]]
                )
                nc.sync.dma_start(
                    x0_ps[g][:], x_d[:, 0, xk0_first[g] : xk0_first[g] + XPIECES[g]]
                )

            for tt in range(N_TILES):
                if tt == 0:
                    x_ps = x0_ps
                    xstarts = xk0_first
                else:
                    x_ps = []
                    for g in range(len(XG_STEADY)):
                        x_p = xpool.tile(
                            [128, XG_STEADY[g], 128], f32, tag=f"x{g}", name=f"xp{g}"
                        )
                        nc.sync.dma_start(
                            x_p[:],
                            x_d[:, tt, xk0_steady[g] : xk0_steady[g] + XG_STEADY[g]],
                        )
                        x_ps.append(x_p)
                    xstarts = xk0_steady

                ps = ppool.tile([128, E], f32, tag="ps")
                for k in range(KC):
                    xi, xo = piece_of(k, xstarts)
                    wi, wo = piece_of(k, wk0)
                    nc.tensor.matmul(
                        ps[:],
                        lhsT=x_ps[xi][:, xo],
                        rhs=w_gs[wi][:, wo],
                        start=(k == 0),
                        stop=(k == KC - 1),
                    )

                # g = sigmoid(scores) = 1/(1+exp(-z)), decomposed exactly as
                # XLA lowers logistic on this backend (bitwise-matching the
                # reference selection): ACT Exp(scale=-1) -> +1 -> DVE recip.
                ex = wpool.tile([128, E], f32, tag="ex")
                nc.scalar.activation(ex[:], ps[:], mybir.ActivationFunctionType.Exp, scale=-1.0)
                u = wpool.tile([128, E], f32, tag="u")
                nc.vector.tensor_scalar(u[:], ex[:], 1.0, None, op0=Alu.add)
                g = wpool.tile([128, E], f32, tag="g")
                nc.vector.reciprocal(g[:], u[:])
                s = wpool.tile([128, E], f32, tag="s")
                nc.vector.tensor_add(s[:], g[:], bias_sb[:])

                # grouped top-2 sums -> group scores [128, 8]
                s3 = s[:].rearrange("p (g e) -> p g e", g=N_GROUPS)
                m1 = spool.tile([128, N_GROUPS], f32, tag="m1")
                nc.vector.tensor_reduce(m1[:], s3, axis=X, op=Alu.max)
                s2 = wpool.tile([128, E], f32, tag="s2")
                nc.vector.match_replace(
                    out=s2[:], in_to_replace=m1[:], in_values=s[:], imm_value=-1e30
                )
                m2 = spool.tile([128, N_GROUPS], f32, tag="m2")
                nc.vector.tensor_reduce(
                    m2[:], s2[:].rearrange("p (g e) -> p g e", g=N_GROUPS), axis=X, op=Alu.max
                )
                gs = spool.tile([128, N_GROUPS], f32, tag="gs")
                nc.vector.tensor_add(gs[:], m1[:], m2[:])

                # top-4 groups: threshold = 4th largest group score
                g8 = spool.tile([128, 8], f32, tag="g8")
                nc.vector.max(g8[:], gs[:])
                gmask = spool.tile([128, N_GROUPS], f32, tag="gmask")
                nc.vector.tensor_scalar(
                    gmask[:], gs[:], g8[:, TOPK_GROUPS - 1 : TOPK_GROUPS], None, op0=Alu.is_ge
                )

                # s_sel = s * gmask (zeros outside selected groups)
                s_sel = wpool.tile([128, E], f32, tag="ssel")
                nc.vector.tensor_tensor(
                    s_sel[:].rearrange("p (g e) -> p g e", g=N_GROUPS),
                    s3,
                    gmask[:].to_broadcast([128, N_GROUPS, EPG]),
                    op=Alu.mult,
                )

                # top-8 experts by biased score
                top8 = spool.tile([128, 8], f32, tag="top8")
                nc.vector.max(top8[:], s_sel[:])
                idx = spool.tile([128, 8], u32, tag="idx")
                nc.vector.max_index(idx[:], top8[:], s_sel[:])

                # positions of the top-8 -> gather sigmoid values (unbiased):
                # z = (s_sel >= 8th_largest) * g in one fused op
                z = wpool.tile([128, E], f32, tag="z")
                nc.vector.scalar_tensor_tensor(
                    z[:], s_sel[:], top8[:, 7:8], g[:], op0=Alu.is_ge, op1=Alu.mult
                )
                z8 = spool.tile([128, 8], f32, tag="z8")
                nc.vector.max(z8[:], z[:])
                zidx = spool.tile([128, 8], u32, tag="zidx")
                nc.vector.max_index(zidx[:], z8[:], z[:])

                # align sigmoid values to the biased-score rank order:
                # w8[p, j] = sum_k (idx[p,j] == zidx[p,k]) * z8[p,k]
                idxf = spool.tile([128, 8], f32, tag="idxf")
                nc.vector.tensor_copy(idxf[:], idx[:])
                zidxf = spool.tile([128, 8], f32, tag="zidxf")
                nc.vector.tensor_copy(zidxf[:], zidx[:])
                eq = spool.tile([128, 8, 8], f32, tag="eq")
                nc.vector.tensor_tensor(
                    eq[:],
                    idxf[:].unsqueeze(2).broadcast_to([128, 8, 8]),
                    zidxf[:].unsqueeze(1).broadcast_to([128, 8, 8]),
                    op=Alu.is_equal,
                )
                wm = spool.tile([128, 8, 8], f32, tag="wm")
                nc.vector.tensor_tensor(
                    wm[:], eq[:], z8[:].unsqueeze(1).broadcast_to([128, 8, 8]), op=Alu.mult
                )
                w8 = spool.tile([128, 8], f32, tag="w8")
                nc.vector.tensor_reduce(w8[:], wm[:], axis=X, op=Alu.add)

                # normalize: out = w8 * (2.5 / (sum(w8) + 1e-20))
                den = spool.tile([128, 1], f32, tag="den")
                nc.vector.tensor_reduce(den[:], w8[:], axis=X, op=Alu.add)
                nc.vector.tensor_scalar(
                    den[:], den[:], 1e-20, 1.0 / SCALE, op0=Alu.add, op1=Alu.mult
                )
                rec = spool.tile([128, 1], f32, tag="rec")
                nc.vector.reciprocal(rec[:], den[:])
                wout = spool.tile([128, 8], f32, tag="wout")
                nc.vector.tensor_scalar(wout[:], w8[:], rec[:], None, op0=Alu.mult)

                nc.sync.dma_start(ow_d[tt], wout[:])
                nc.sync.dma_start(oi_d[tt], idx[:])

    nc.finalize()
    return nc


def _get_nc():
    if "nc" not in _CACHE:
        _CACHE["nc"] = _build_nc()
    return _CACHE["nc"]


def _prep_inputs(x_TD, kernel_DE, bias_E):
    # w layout: w_sb[p, k, e] = kernel_DE[k*128 + p, e]
    w_l = np.ascontiguousarray(
        kernel_DE.reshape(KC, 128, E).transpose(1, 0, 2)
    )
    bias_rep = np.ascontiguousarray(np.tile(bias_E[None, :], (128, 1)))
    in_maps = []
    for c in range(N_CORES):
        xc = x_TD[c * TPC : (c + 1) * TPC]  # [1024, 7168]
        # x_sb[p, tt, k, t] = xc[tt*128 + t, k*128 + p]
        xl = np.ascontiguousarray(
            xc.reshape(N_TILES, 128, KC, 128).transpose(3, 0, 2, 1)
        )
        in_maps.append({"x": xl, "w": w_l, "bias": bias_rep})
    return in_maps


def kernel(x_TD, kernel_DE, bias_E, _trace=False):
    from concourse import bass_utils

    x_TD = np.asarray(x_TD, dtype=np.float32)
    kernel_DE = np.asarray(kernel_DE, dtype=np.float32)
    bias_E = np.asarray(bias_E, dtype=np.float32)

    nc = _get_nc()
    in_maps = _prep_inputs(x_TD, kernel_DE, bias_E)
    res = bass_utils.run_bass_kernel_spmd(
        nc, in_maps, core_ids=list(range(N_CORES)), trace=_trace
    )
    _CACHE["last_results"] = res
    weights = np.concatenate(
        [res.results[c]["out_w"].reshape(TPC, TOP_K) for c in range(N_CORES)], axis=0
    )
    indices = np.concatenate(
        [
            res.results[c]["out_idx"].reshape(TPC, TOP_K).astype(np.int32)
            for c in range(N_CORES)
        ],
        axis=0,
    )
    return weights, indices


if __name__ == "__main__":
    rng = np.random.default_rng(0)
    x = rng.standard_normal((T, D), dtype=np.float32)
    w = rng.standard_normal((D, E), dtype=np.float32) / np.sqrt(D)
    b = (rng.standard_normal(E) * 0.01).astype(np.float32)
    wts, idx = kernel(x, w, b)
    print("weights", wts.shape, wts.dtype, "indices", idx.shape, idx.dtype)
    print(wts[:2])
    print(idx[:2])



# revision 3
# speedup vs baseline: 1.3847x; 1.3847x over previous
"""DeepSeekV3 MoE router on 8 TRN2 NeuronCores (Bass/Tile).

Strategy (hardcoded for T=8192, D=7168, E=256, top-k=8, 8 groups, top-4 groups):
  - Data-parallel over tokens: each of 8 cores handles 1024 tokens.
  - Matmul runs in fp16 at 1 cycle/row (4x the fp32 rate) using an
    error-compensated split: x = xh + xl/2048, w = wh + wl/2048 (all
    fp16; the lo parts are scaled by 2^11 so they stay in fp16 normal
    range).  z = xh.wh + (xl.wh + xh.wl)/2048 captures ~22 bits of
    both operands; the dropped (xl.wl)/2048^2 term is ~2^-22.
  - Per 128-token tile: one N=512 accumulating pass computes
    [xh.wh | xh.wl] into a single PSUM bank (56 matmuls), one N=256
    pass computes xl.wh (56 matmuls), then two DVE ops combine:
    z = hh + (hl + lh)/2048.
  - Selection (grouped top-2 / top-4-groups / top-8) identical to the
    proven fp32 baseline pipeline.
"""

import sys

for p in ("/opt/trn_rl_repo", "/root/.axon_site/_ro/trn_rl_repo"):
    if p not in sys.path:
        sys.path.insert(0, p)

import numpy as np

T = 8192
D = 7168
E = 256
TOP_K = 8
N_GROUPS = 8
EPG = E // N_GROUPS  # experts per group = 32
TOPK_GROUPS = 4
SCALE = 2.5
N_CORES = 8
TPC = T // N_CORES  # tokens per core = 1024
N_TILES = TPC // 128  # 8 token tiles per core
KC = D // 128  # 56 contraction chunks

LO_SCALE = 2048.0  # 2^11: lo-part pre-scale (keeps fp16 normal range)

_CACHE = {}


def _build_nc():
    import concourse.bacc as bacc
    import concourse.mybir as mybir
    import concourse.tile as tile

    f32 = mybir.dt.float32
    f16 = mybir.dt.float16
    u32 = mybir.dt.uint32
    X = mybir.AxisListType.X
    Alu = mybir.AluOpType

    nc = bacc.Bacc(trn_type="TRN2")
    xh_d = nc.declare_dram_parameter("xh", [128, N_TILES, KC, 128], f16, isOutput=False)
    xl_d = nc.declare_dram_parameter("xl", [128, N_TILES, KC, 128], f16, isOutput=False)
    # w2[:, k, 0:256] = wh chunk k, w2[:, k, 256:512] = wl chunk k
    w2_d = nc.declare_dram_parameter("w2", [128, KC, 2 * E], f16, isOutput=False)
    b_d = nc.declare_dram_parameter("bias", [128, E], f32, isOutput=False)
    ow_d = nc.declare_dram_parameter("out_w", [N_TILES, 128, TOP_K], f32, isOutput=True)
    oi_d = nc.declare_dram_parameter("out_idx", [N_TILES, 128, TOP_K], u32, isOutput=True)

    with tile.TileContext(nc) as tc:
        with (
            tc.tile_pool(name="const", bufs=1) as cpool,
            tc.tile_pool(name="xin", bufs=2) as xpool,
            tc.tile_pool(name="xfirst", bufs=1) as xfpool,
            tc.tile_pool(name="work", bufs=2) as wpool,
            tc.tile_pool(name="small", bufs=2) as spool,
            tc.tile_pool(name="psum", bufs=2, space="PSUM") as ppool,
        ):
            # Graduated piece sizes: tiny leading pieces get the PE started
            # early while the DMA streams the rest in k-consumption order.
            WPIECES = [2, 2, 3, 7, 14, 14, 14]  # sums to KC=56
            XPIECES = [2, 2, 3, 7, 14, 14, 14]
            XG_STEADY = [14, 14, 14, 14]
            wk0 = [sum(WPIECES[:i]) for i in range(len(WPIECES))]
            xk0_first = [sum(XPIECES[:i]) for i in range(len(XPIECES))]
            xk0_steady = [sum(XG_STEADY[:i]) for i in range(len(XG_STEADY))]

            def piece_of(k, starts):
                for i in range(len(starts) - 1, -1, -1):
                    if k >= starts[i]:
                        return i, k - starts[i]
                raise AssertionError

            bias_sb = cpool.tile([128, E], f32)
            nc.sync.dma_start(bias_sb[:], b_d[:])
            w_gs = [
                cpool.tile([128, WPIECES[g], 2 * E], f16, tag=f"w{g}", name=f"w{g}")
                for g in range(len(WPIECES))
            ]
            xh0_ps = [
                xfpool.tile([128, XPIECES[g], 128], f16, tag=f"xfh{g}", name=f"x0h{g}")
                for g in range(len(XPIECES))
            ]
            xl0_ps = [
                xfpool.tile([128, XPIECES[g], 128], f16, tag=f"xfl{g}", name=f"x0l{g}")
                for g in range(len(XPIECES))
            ]
            # Interleave W / tile-0 x pieces in k-consumption order.
            for g in range(len(WPIECES)):
                nc.sync.dma_start(
                    w_gs[g][:], w2_d[:, wk0[g] : wk0[g] + WPIECES[g]]
                )
                nc.sync.dma_start(
                    xh0_ps[g][:], xh_d[:, 0, xk0_first[g] : xk0_first[g] + XPIECES[g]]
                )
            for g in range(len(XPIECES)):
                nc.sync.dma_start(
                    xl0_ps[g][:], xl_d[:, 0, xk0_first[g] : xk0_first[g] + XPIECES[g]]
                )

            for tt in range(N_TILES):
                if tt == 0:
                    xh_ps, xl_ps = xh0_ps, xl0_ps
                    xstarts = xk0_first
                else:
                    xh_ps, xl_ps = [], []
                    for g in range(len(XG_STEADY)):
                        xh_p = xpool.tile(
                            [128, XG_STEADY[g], 128], f16, tag=f"xh{g}", name=f"xph{g}"
                        )
                        nc.sync.dma_start(
                            xh_p[:],
                            xh_d[:, tt, xk0_steady[g] : xk0_steady[g] + XG_STEADY[g]],
                        )
                        xh_ps.append(xh_p)
                        xl_p = xpool.tile(
                            [128, XG_STEADY[g], 128], f16, tag=f"xl{g}", name=f"xpl{g}"
                        )
                        nc.sync.dma_start(
                            xl_p[:],
                            xl_d[:, tt, xk0_steady[g] : xk0_steady[g] + XG_STEADY[g]],
                        )
                        xl_ps.append(xl_p)
                    xstarts = xk0_steady

                # Pass A (N=512): psW = [sum_k xh.wh | sum_k xh.wl]
                psW = ppool.tile([128, 2 * E], f32, tag="psW")
                for k in range(KC):
                    xi, xo = piece_of(k, xstarts)
                    wi, wo = piece_of(k, wk0)
                    nc.tensor.matmul(
                        psW[:],
                        lhsT=xh_ps[xi][:, xo],
                        rhs=w_gs[wi][:, wo],
                        start=(k == 0),
                        stop=(k == KC - 1),
                    )
                # Pass B (N=256): psB = sum_k xl.wh
                psB = ppool.tile([128, E], f32, tag="psB")
                for k in range(KC):
                    xi, xo = piece_of(k, xstarts)
                    wi, wo = piece_of(k, wk0)
                    nc.tensor.matmul(
                        psB[:],
                        lhsT=xl_ps[xi][:, xo],
                        rhs=w_gs[wi][:, wo, 0:E],
                        start=(k == 0),
                        stop=(k == KC - 1),
                    )

                # z = hh + (hl + lh)/2048  (DVE reads at most one PSUM input
                # per op, so: copy hl out, add lh, then scale-add to hh)
                t1 = wpool.tile([128, E], f32, tag="t1")
                nc.vector.tensor_copy(t1[:], psW[:, E : 2 * E])
                t2 = wpool.tile([128, E], f32, tag="t2")
                nc.vector.tensor_tensor(t2[:], psB[:], t1[:], op=Alu.add)
                z = wpool.tile([128, E], f32, tag="z")
                nc.vector.scalar_tensor_tensor(
                    z[:], t2[:], 1.0 / LO_SCALE, psW[:, 0:E],
                    op0=Alu.mult, op1=Alu.add,
                )

                # g = sigmoid(z) decomposed exactly as XLA lowers logistic:
                # ACT Exp(scale=-1) -> +1 -> DVE recip.
                ex = wpool.tile([128, E], f32, tag="ex")
                nc.scalar.activation(ex[:], z[:], mybir.ActivationFunctionType.Exp, scale=-1.0)
                u = wpool.tile([128, E], f32, tag="u")
                nc.vector.tensor_scalar(u[:], ex[:], 1.0, None, op0=Alu.add)
                g = wpool.tile([128, E], f32, tag="g")
                nc.vector.reciprocal(g[:], u[:])
                s = wpool.tile([128, E], f32, tag="s")
                nc.vector.tensor_add(s[:], g[:], bias_sb[:])

                # grouped top-2 sums -> group scores [128, 8]
                s3 = s[:].rearrange("p (g e) -> p g e", g=N_GROUPS)
                m1 = spool.tile([128, N_GROUPS], f32, tag="m1")
                nc.vector.tensor_reduce(m1[:], s3, axis=X, op=Alu.max)
                s2 = wpool.tile([128, E], f32, tag="s2")
                nc.vector.match_replace(
                    out=s2[:], in_to_replace=m1[:], in_values=s[:], imm_value=-1e30
                )
                m2 = spool.tile([128, N_GROUPS], f32, tag="m2")
                nc.vector.tensor_reduce(
                    m2[:], s2[:].rearrange("p (g e) -> p g e", g=N_GROUPS), axis=X, op=Alu.max
                )
                gs = spool.tile([128, N_GROUPS], f32, tag="gs")
                nc.vector.tensor_add(gs[:], m1[:], m2[:])

                # top-4 groups: threshold = 4th largest group score
                g8 = spool.tile([128, 8], f32, tag="g8")
                nc.vector.max(g8[:], gs[:])
                gmask = spool.tile([128, N_GROUPS], f32, tag="gmask")
                nc.vector.tensor_scalar(
                    gmask[:], gs[:], g8[:, TOPK_GROUPS - 1 : TOPK_GROUPS], None, op0=Alu.is_ge
                )

                # s_sel = s * gmask (zeros outside selected groups)
                s_sel = wpool.tile([128, E], f32, tag="ssel")
                nc.vector.tensor_tensor(
                    s_sel[:].rearrange("p (g e) -> p g e", g=N_GROUPS),
                    s3,
                    gmask[:].to_broadcast([128, N_GROUPS, EPG]),
                    op=Alu.mult,
                )

                # top-8 experts by biased score
                top8 = spool.tile([128, 8], f32, tag="top8")
                nc.vector.max(top8[:], s_sel[:])
                idx = spool.tile([128, 8], u32, tag="idx")
                nc.vector.max_index(idx[:], top8[:], s_sel[:])

                # positions of the top-8 -> gather sigmoid values (unbiased):
                # z = (s_sel >= 8th_largest) * g in one fused op
                zt = wpool.tile([128, E], f32, tag="zt")
                nc.vector.scalar_tensor_tensor(
                    zt[:], s_sel[:], top8[:, 7:8], g[:], op0=Alu.is_ge, op1=Alu.mult
                )
                z8 = spool.tile([128, 8], f32, tag="z8")
                nc.vector.max(z8[:], zt[:])
                zidx = spool.tile([128, 8], u32, tag="zidx")
                nc.vector.max_index(zidx[:], z8[:], zt[:])

                # align sigmoid values to the biased-score rank order:
                # w8[p, j] = sum_k (idx[p,j] == zidx[p,k]) * z8[p,k]
                idxf = spool.tile([128, 8], f32, tag="idxf")
                nc.vector.tensor_copy(idxf[:], idx[:])
                zidxf = spool.tile([128, 8], f32, tag="zidxf")
                nc.vector.tensor_copy(zidxf[:], zidx[:])
                eq = spool.tile([128, 8, 8], f32, tag="eq")
                nc.vector.tensor_tensor(
                    eq[:],
                    idxf[:].unsqueeze(2).broadcast_to([128, 8, 8]),
                    zidxf[:].unsqueeze(1).broadcast_to([128, 8, 8]),
                    op=Alu.is_equal,
                )
                wm = spool.tile([128, 8, 8], f32, tag="wm")
                nc.vector.tensor_tensor(
                    wm[:], eq[:], z8[:].unsqueeze(1).broadcast_to([128, 8, 8]), op=Alu.mult
                )
                w8 = spool.tile([128, 8], f32, tag="w8")
                nc.vector.tensor_reduce(w8[:], wm[:], axis=X, op=Alu.add)

                # normalize: out = w8 * (2.5 / (sum(w8) + 1e-20))
                den = spool.tile([128, 1], f32, tag="den")
                nc.vector.tensor_reduce(den[:], w8[:], axis=X, op=Alu.add)
                nc.vector.tensor_scalar(
                    den[:], den[:], 1e-20, 1.0 / SCALE, op0=Alu.add, op1=Alu.mult
                )
                rec = spool.tile([128, 1], f32, tag="rec")
                nc.vector.reciprocal(rec[:], den[:])
                wout = spool.tile([128, 8], f32, tag="wout")
                nc.vector.tensor_scalar(wout[:], w8[:], rec[:], None, op0=Alu.mult)

                nc.sync.dma_start(ow_d[tt], wout[:])
                nc.sync.dma_start(oi_d[tt], idx[:])

    nc.finalize()
    return nc


def _get_nc():
    if "nc" not in _CACHE:
        _CACHE["nc"] = _build_nc()
    return _CACHE["nc"]


def _split_f16(a32):
    """a32 (np.float32) -> (hi, lo) fp16 with lo pre-scaled by 2^11."""
    hi = a32.astype(np.float16)
    lo = ((a32 - hi.astype(np.float32)) * np.float32(LO_SCALE)).astype(np.float16)
    return hi, lo


def _prep_inputs(x_TD, kernel_DE, bias_E):
    wh, wl = _split_f16(kernel_DE)
    # w2[p, k, 0:E] = wh[k*128+p, e]; w2[p, k, E:2E] = wl[k*128+p, e]
    w2 = np.concatenate(
        [wh.reshape(KC, 128, E), wl.reshape(KC, 128, E)], axis=2
    ).transpose(1, 0, 2)
    w2 = np.ascontiguousarray(w2)
    bias_rep = np.ascontiguousarray(np.tile(bias_E[None, :], (128, 1)))
    in_maps = []
    for c in range(N_CORES):
        xc = x_TD[c * TPC : (c + 1) * TPC]  # [1024, 7168]
        xh, xl = _split_f16(xc)
        # x_sb[p, tt, k, t] = xc[tt*128 + t, k*128 + p]
        xh_l = np.ascontiguousarray(
            xh.reshape(N_TILES, 128, KC, 128).transpose(3, 0, 2, 1)
        )
        xl_l = np.ascontiguousarray(
            xl.reshape(N_TILES, 128, KC, 128).transpose(3, 0, 2, 1)
        )
        in_maps.append({"xh": xh_l, "xl": xl_l, "w2": w2, "bias": bias_rep})
    return in_maps


def kernel(x_TD, kernel_DE, bias_E, _trace=False):
    from concourse import bass_utils

    x_TD = np.asarray(x_TD, dtype=np.float32)
    kernel_DE = np.asarray(kernel_DE, dtype=np.float32)
    bias_E = np.asarray(bias_E, dtype=np.float32)

    nc = _get_nc()
    in_maps = _prep_inputs(x_TD, kernel_DE, bias_E)
    res = bass_utils.run_bass_kernel_spmd(
        nc, in_maps, core_ids=list(range(N_CORES)), trace=_trace
    )
    _CACHE["last_results"] = res
    weights = np.concatenate(
        [res.results[c]["out_w"].reshape(TPC, TOP_K) for c in range(N_CORES)], axis=0
    )
    indices = np.concatenate(
        [
            res.results[c]["out_idx"].reshape(TPC, TOP_K).astype(np.int32)
            for c in range(N_CORES)
        ],
        axis=0,
    )
    return weights, indices


if __name__ == "__main__":
    rng = np.random.default_rng(0)
    x = rng.standard_normal((T, D), dtype=np.float32)
    w = rng.standard_normal((D, E), dtype=np.float32) / np.sqrt(D)
    b = (rng.standard_normal(E) * 0.01).astype(np.float32)
    wts, idx = kernel(x, w, b)
    print("weights", wts.shape, wts.dtype, "indices", idx.shape, idx.dtype)
    print(wts[:2])
    print(idx[:2])


# revision 6
# speedup vs baseline: 1.5005x; 1.0837x over previous
"""DeepSeekV3 MoE router on 8 TRN2 NeuronCores (Bass/Tile).

Strategy (hardcoded for T=8192, D=7168, E=256, top-k=8, 8 groups, top-4 groups):
  - Data-parallel over tokens: each of 8 cores handles 1024 tokens.
  - Matmul runs in fp16 at 1 cycle/row (4x the fp32 rate) using an
    error-compensated split: x = xh + xl/2048, w = wh + wl/2048 (all
    fp16; lo parts pre-scaled by 2^11 to stay in fp16 normal range).
    z = xh.wh + (xh.wl + xl.wh)/2048 captures ~22 bits of both
    operands (dropped xl.wl term is ~2^-22).
  - Per 128-token tile, ONE PSUM bank [128,512]:
      wide pass  (N=512): psW += xh_k . [wh_k | wl_k]   (56 matmuls)
      narrow pass (N=256): psW[:,256:512] += xl_k . wh_k (56 matmuls)
    then z = psW[:,0:256] + psW[:,256:512]/2048 (2 DVE ops).
  - Tiles 0-3 run their wide passes k-interleaved so each W chunk is
    used 4x on arrival: the 7.3MB weight load streams at ~307GB/s
    demand instead of 1.4TB/s, keeping the PE dense (and the HAM
    clock-gate warm) from the start.
  - Selection: sigmoid via ACT Exp + DVE add/recip (bitwise identical
    to XLA logistic), grouped top-2 via MAX8, top-4 groups by
    threshold, top-8 by biased score, weight gather via compare/align.
"""

import sys

for p in ("/opt/trn_rl_repo", "/root/.axon_site/_ro/trn_rl_repo"):
    if p not in sys.path:
        sys.path.insert(0, p)

import numpy as np

T = 8192
D = 7168
E = 256
TOP_K = 8
N_GROUPS = 8
EPG = E // N_GROUPS  # experts per group = 32
TOPK_GROUPS = 4
SCALE = 2.5
N_CORES = 8
TPC = T // N_CORES  # tokens per core = 1024
N_TILES = TPC // 128  # 8 token tiles per core
KC = D // 128  # 56 contraction chunks
NFIRST = 4  # tiles with k-interleaved wide passes at startup

LO_SCALE = 2048.0  # 2^11: lo-part pre-scale (keeps fp16 normal range)

_CACHE = {}


def _build_nc():
    import concourse.bacc as bacc
    import concourse.mybir as mybir
    import concourse.tile as tile

    f32 = mybir.dt.float32
    f16 = mybir.dt.float16
    u32 = mybir.dt.uint32
    X = mybir.AxisListType.X
    Alu = mybir.AluOpType

    nc = bacc.Bacc(trn_type="TRN2")
    xh_d = nc.declare_dram_parameter("xh", [128, N_TILES, KC, 128], f16, isOutput=False)
    xl_d = nc.declare_dram_parameter("xl", [128, N_TILES, KC, 128], f16, isOutput=False)
    # w2[:, k, 0:256] = wh chunk k, w2[:, k, 256:512] = wl chunk k
    w2_d = nc.declare_dram_parameter("w2", [128, KC, 2 * E], f16, isOutput=False)
    b_d = nc.declare_dram_parameter("bias", [128, E], f32, isOutput=False)
    ow_d = nc.declare_dram_parameter("out_w", [N_TILES, 128, TOP_K], f32, isOutput=True)
    oi_d = nc.declare_dram_parameter("out_idx", [N_TILES, 128, TOP_K], u32, isOutput=True)

    with tile.TileContext(nc) as tc:
        with (
            tc.tile_pool(name="const", bufs=1) as cpool,
            tc.tile_pool(name="xin", bufs=2) as xpool,
            tc.tile_pool(name="xfirst", bufs=1) as xfpool,
            tc.tile_pool(name="work", bufs=2) as wpool,
            tc.tile_pool(name="small", bufs=2) as spool,
            tc.tile_pool(name="psum", bufs=1, space="PSUM") as ppool,
        ):
            WPIECES = [2, 2, 3, 7, 14, 14, 14]  # sums to KC=56
            wk0 = [sum(WPIECES[:i]) for i in range(len(WPIECES))]

            def piece_of(k, starts, pieces):
                for i in range(len(starts) - 1, -1, -1):
                    if k >= starts[i]:
                        return i, k - starts[i]
                raise AssertionError

            bias_sb = cpool.tile([128, E], f32)
            nc.sync.dma_start(bias_sb[:], b_d[:])
            w_gs = [
                cpool.tile([128, WPIECES[g], 2 * E], f16, tag=f"w{g}", name=f"w{g}")
                for g in range(len(WPIECES))
            ]
            # first tiles: graduated xh pieces, interleaved with W in
            # k-consumption order
            xh0_ps = [
                [
                    xfpool.tile(
                        [128, WPIECES[g], 128], f16, tag=f"xf{t}_{g}", name=f"xf{t}_{g}"
                    )
                    for g in range(len(WPIECES))
                ]
                for t in range(NFIRST)
            ]
            for g in range(len(WPIECES)):
                nc.sync.dma_start(w_gs[g][:], w2_d[:, wk0[g] : wk0[g] + WPIECES[g]])
                for t in range(NFIRST):
                    nc.sync.dma_start(
                        xh0_ps[t][g][:],
                        xh_d[:, t, wk0[g] : wk0[g] + WPIECES[g]],
                    )

            psWs = [
                ppool.tile([128, 2 * E], f32, tag=f"psW{i}", name=f"psW{i}")
                for i in range(NFIRST)
            ]

            # Phase 1: wide passes of tiles 0..3, k-interleaved.
            for k in range(KC):
                wi, wo = piece_of(k, wk0, WPIECES)
                for t in range(NFIRST):
                    nc.tensor.matmul(
                        psWs[t][:],
                        lhsT=xh0_ps[t][wi][:, wo],
                        rhs=w_gs[wi][:, wo],
                        start=(k == 0),
                        stop=False,
                        skip_group_check=True,
                    )

            def narrow_pass(psW, xl_t):
                """psW[:,E:2E] += sum_k xl_k . wh_k; ends the psW group."""
                for k in range(KC):
                    wi, wo = piece_of(k, wk0, WPIECES)
                    nc.tensor.matmul(
                        psW[:, E : 2 * E],
                        lhsT=xl_t[:, k],
                        rhs=w_gs[wi][:, wo, 0:E],
                        start=False,
                        stop=(k == KC - 1),
                        skip_group_check=True,
                    )

            def wide_pass(psW, xh_t):
                for k in range(KC):
                    wi, wo = piece_of(k, wk0, WPIECES)
                    nc.tensor.matmul(
                        psW[:],
                        lhsT=xh_t[:, k],
                        rhs=w_gs[wi][:, wo],
                        start=(k == 0),
                        stop=False,
                        skip_group_check=True,
                    )

            def selection(tt, psW):
                # z = hh + (hl + lh)/2048
                t1 = wpool.tile([128, E], f32, tag="t1")
                nc.vector.tensor_scalar(
                    t1[:], psW[:, E : 2 * E], 1.0 / LO_SCALE, None, op0=Alu.mult
                )
                z = wpool.tile([128, E], f32, tag="z")
                nc.vector.tensor_tensor(z[:], psW[:, 0:E], t1[:], op=Alu.add)

                # g = sigmoid(z) = 1/(1+exp(-z)), decomposed exactly as XLA
                # lowers logistic (bitwise-matching the reference selection)
                ex = wpool.tile([128, E], f32, tag="ex")
                nc.scalar.activation(
                    ex[:], z[:], mybir.ActivationFunctionType.Exp, scale=-1.0
                )
                u = wpool.tile([128, E], f32, tag="u")
                nc.vector.tensor_scalar(u[:], ex[:], 1.0, None, op0=Alu.add)
                g = wpool.tile([128, E], f32, tag="g")
                nc.vector.reciprocal(g[:], u[:])
                s = wpool.tile([128, E], f32, tag="s")
                nc.vector.tensor_add(s[:], g[:], bias_sb[:])

                # group scores: top-2 per group of 32, summed -> [128, 8]
                s3 = s[:].rearrange("p (g e) -> p g e", g=N_GROUPS)
                m1 = spool.tile([128, N_GROUPS], f32, tag="m1")
                nc.vector.tensor_reduce(m1[:], s3, axis=X, op=Alu.max)
                s2 = wpool.tile([128, E], f32, tag="s2")
                nc.vector.match_replace(
                    out=s2[:], in_to_replace=m1[:], in_values=s[:], imm_value=-1e30
                )
                m2 = spool.tile([128, N_GROUPS], f32, tag="m2")
                nc.vector.tensor_reduce(
                    m2[:], s2[:].rearrange("p (g e) -> p g e", g=N_GROUPS),
                    axis=X, op=Alu.max,
                )
                gs = spool.tile([128, N_GROUPS], f32, tag="gs")
                nc.vector.tensor_add(gs[:], m1[:], m2[:])

                # top-4 groups: threshold = 4th largest group score
                g8 = spool.tile([128, 8], f32, tag="g8")
                nc.vector.max(g8[:], gs[:])
                gmask = spool.tile([128, N_GROUPS], f32, tag="gmask")
                nc.vector.tensor_scalar(
                    gmask[:], gs[:], g8[:, TOPK_GROUPS - 1 : TOPK_GROUPS], None,
                    op0=Alu.is_ge,
                )

                # s_sel = s * gmask (zeros outside selected groups)
                s_sel = wpool.tile([128, E], f32, tag="ssel")
                nc.vector.tensor_tensor(
                    s_sel[:].rearrange("p (g e) -> p g e", g=N_GROUPS),
                    s3,
                    gmask[:].to_broadcast([128, N_GROUPS, EPG]),
                    op=Alu.mult,
                )

                # top-8 experts by biased score
                top8 = spool.tile([128, 8], f32, tag="top8")
                nc.vector.max(top8[:], s_sel[:])
                idx = spool.tile([128, 8], u32, tag="idx")
                nc.vector.max_index(idx[:], top8[:], s_sel[:])

                # gather sigmoid values of the top-8 (unbiased):
                # zt = (s_sel >= 8th_largest) * g
                zt = wpool.tile([128, E], f32, tag="zt")
                nc.vector.scalar_tensor_tensor(
                    zt[:], s_sel[:], top8[:, 7:8], g[:], op0=Alu.is_ge, op1=Alu.mult
                )
                z8 = spool.tile([128, 8], f32, tag="z8")
                nc.vector.max(z8[:], zt[:])
                zidx = spool.tile([128, 8], u32, tag="zidx")
                nc.vector.max_index(zidx[:], z8[:], zt[:])

                # align sigmoid values to the biased-score rank order:
                # w8[p, j] = sum_k (idx[p,j] == zidx[p,k]) * z8[p,k]
                eq = spool.tile([128, 8, 8], f32, tag="eq")
                nc.vector.tensor_tensor(
                    eq[:],
                    idx[:].unsqueeze(2).broadcast_to([128, 8, 8]),
                    zidx[:].unsqueeze(1).broadcast_to([128, 8, 8]),
                    op=Alu.is_equal,
                )
                wm = spool.tile([128, 8, 8], f32, tag="wm")
                nc.vector.tensor_tensor(
                    wm[:], eq[:], z8[:].unsqueeze(1).broadcast_to([128, 8, 8]),
                    op=Alu.mult,
                )
                w8 = spool.tile([128, 8], f32, tag="w8")
                nc.vector.tensor_reduce(w8[:], wm[:], axis=X, op=Alu.add)

                # normalize: out = w8 * (2.5 / (sum(w8) + 1e-20))
                den = spool.tile([128, 1], f32, tag="den")
                nc.vector.tensor_reduce(den[:], w8[:], axis=X, op=Alu.add)
                nc.vector.tensor_scalar(
                    den[:], den[:], 1e-20, 1.0 / SCALE, op0=Alu.add, op1=Alu.mult
                )
                rec = spool.tile([128, 1], f32, tag="rec")
                nc.vector.reciprocal(rec[:], den[:])
                wout = spool.tile([128, 8], f32, tag="wout")
                nc.vector.tensor_scalar(wout[:], w8[:], rec[:], None, op0=Alu.mult)

                nc.sync.dma_start(ow_d[tt], wout[:])
                nc.sync.dma_start(oi_d[tt], idx[:])

            # Phase 2: narrow passes + selection for tiles 0..3
            for t in range(NFIRST):
                xl_t = xpool.tile([128, KC, 128], f16, tag="xl", name=f"xl{t}")
                nc.sync.dma_start(xl_t[:], xl_d[:, t])
                narrow_pass(psWs[t], xl_t)
                selection(t, psWs[t])

            # Phase 3: steady tiles 4..7
            for tt in range(NFIRST, N_TILES):
                xh_t = xpool.tile([128, KC, 128], f16, tag="xh", name=f"xh{tt}")
                nc.sync.dma_start(xh_t[:], xh_d[:, tt])
                xl_t = xpool.tile([128, KC, 128], f16, tag="xl", name=f"xl{tt}")
                nc.sync.dma_start(xl_t[:], xl_d[:, tt])
                psW = psWs[tt % NFIRST]
                wide_pass(psW, xh_t)
                narrow_pass(psW, xl_t)
                selection(tt, psW)

    nc.finalize()
    return nc


def _get_nc():
    if "nc" not in _CACHE:
        _CACHE["nc"] = _build_nc()
    return _CACHE["nc"]


def _split_f16(a32):
    """a32 (np.float32) -> (hi, lo) fp16 with lo pre-scaled by 2^11."""
    hi = a32.astype(np.float16)
    lo = ((a32 - hi.astype(np.float32)) * np.float32(LO_SCALE)).astype(np.float16)
    return hi, lo


def _prep_inputs(x_TD, kernel_DE, bias_E):
    wh, wl = _split_f16(kernel_DE)
    # w2[p, k, 0:E] = wh[k*128+p, e]; w2[p, k, E:2E] = wl[k*128+p, e]
    w2 = np.concatenate(
        [wh.reshape(KC, 128, E), wl.reshape(KC, 128, E)], axis=2
    ).transpose(1, 0, 2)
    w2 = np.ascontiguousarray(w2)
    bias_rep = np.ascontiguousarray(np.tile(bias_E[None, :], (128, 1)))
    in_maps = []
    for c in range(N_CORES):
        xc = x_TD[c * TPC : (c + 1) * TPC]  # [1024, 7168]
        xh, xl = _split_f16(xc)
        # x_sb[p, tt, k, t] = xc[tt*128 + t, k*128 + p]
        xh_l = np.ascontiguousarray(
            xh.reshape(N_TILES, 128, KC, 128).transpose(3, 0, 2, 1)
        )
        xl_l = np.ascontiguousarray(
            xl.reshape(N_TILES, 128, KC, 128).transpose(3, 0, 2, 1)
        )
        in_maps.append({"xh": xh_l, "xl": xl_l, "w2": w2, "bias": bias_rep})
    return in_maps


def kernel(x_TD, kernel_DE, bias_E, _trace=False):
    from concourse import bass_utils

    x_TD = np.asarray(x_TD, dtype=np.float32)
    kernel_DE = np.asarray(kernel_DE, dtype=np.float32)
    bias_E = np.asarray(bias_E, dtype=np.float32)

    nc = _get_nc()
    in_maps = _prep_inputs(x_TD, kernel_DE, bias_E)
    res = bass_utils.run_bass_kernel_spmd(
        nc, in_maps, core_ids=list(range(N_CORES)), trace=_trace
    )
    _CACHE["last_results"] = res
    weights = np.concatenate(
        [res.results[c]["out_w"].reshape(TPC, TOP_K) for c in range(N_CORES)], axis=0
    )
    indices = np.concatenate(
        [
            res.results[c]["out_idx"].reshape(TPC, TOP_K).astype(np.int32)
            for c in range(N_CORES)
        ],
        axis=0,
    )
    return weights, indices


if __name__ == "__main__":
    rng = np.random.default_rng(0)
    x = rng.standard_normal((T, D), dtype=np.float32)
    w = rng.standard_normal((D, E), dtype=np.float32) / np.sqrt(D)
    b = (rng.standard_normal(E) * 0.01).astype(np.float32)
    wts, idx = kernel(x, w, b)
    print("weights", wts.shape, wts.dtype, "indices", idx.shape, idx.dtype)
    print(wts[:2])
    print(idx[:2])
